# revision 63
# baseline (speedup 1.0000x reference)
"""Trainium2 Bass kernel for nn_Model4 (retrieval_knn).

Model: 3 l2-normalized feature streams -> 4 chained MultiheadAttention blocks
-> full = rt @ t_r.T -> per-group cosine logits [4, 256, 256].

Sharding (8 cores = 4 row-groups x 2 head-halves): core c = 2*g + j.
g owns rows R_g = [256g, 256g+256) (== final group g); j owns qkv feature
columns [512j, 512j+512) for the ff/rt MHAs.

The modeled collective cost (15us constant + bytes/40GBps, serialized on one
device) dominates, so the design minimizes collectives and overlaps them:
  - tl/tg MHAs: fully replicated within the pair (all 4 heads; K/V computed
    locally over the full sequence).  No collective at all for these two;
    the tl chain runs on the PE under the first AllGather.
  - t_r computed in full on every core; B = wo_rt . t_r and g = t_r.T bo_rt
    precomputed during the AG_E window so the tail is full.T = B.T@ctx + g
    with no rt out-projection after the last gather.
  - ff/rt MHAs: head-half sharded; K/V S-sharded + quad AllGather (K fp8,
    V bf16); context halves pair AllGather.  => 4 collectives total.

Mixed precision: fp8e4 (x16) with DoubleRow matmuls on the Q/K projections
and attention scores everywhere -- the softmax is insensitive there.  All
V/context/out-projection paths stay bf16 (attention outputs are ~0.7x the
residual stream, so fp8 there would put ~4-6% on the logits).  fp32 PSUM
accumulation throughout.

Parameter-only host folds: the K-side pos-embedding term is dropped (adds a
per-query constant to the scores, softmax-invariant), the V-side pos term
and the whole V bias are pushed through the out-projection into its bias
(bo' = bo + (bv + pos@wv.T) @ wo.T); the K projection bias is dropped for
the same softmax-invariance reason; q biases ship x256 so the fp8 qp
(ps + bq)/16 copy-out fuses into one op.  The K projection runs on the raw
x16 fp8 stream and the l2-norm column scaling commutes to its output.

Attention runs heads in interleaved pairs sharing one reciprocal and one
broadcast so cross-engine latencies hide under the other head's matmuls.
The final cosine normalization is folded into the logits copy-out as a
per-partition scale (inverse norms transposed via a tiny DRAM roundtrip).
"""
import os
import sys

sys.path.insert(0, "/opt/trn_rl_repo")

import ml_dtypes
import numpy as np

import concourse.bass as bass  # noqa: F401
import concourse.tile as tile
import concourse.mybir as mybir
from concourse import bacc
from concourse.bass_utils import run_bass_kernel_spmd

E = 1024
P = 128
KO = E // P          # 8 feature chunks
RG = 256             # rows per group
NCORES = 8
F32 = mybir.dt.float32
F32R = mybir.dt.float32r
BF16 = mybir.dt.bfloat16
FP8 = mybir.dt.float8e4
U8 = mybir.dt.uint8
DRow = mybir.MatmulPerfMode.DoubleRow
AF = mybir.ActivationFunctionType
GROUPS4 = [[0, 2, 4, 6], [1, 3, 5, 7]]   # gather S-shards across row-groups
GROUPS2 = [[0, 1], [2, 3], [4, 5], [6, 7]]  # exchange head halves within pair
EPS = 1e-8
S16 = 16.0                   # fp8 scale
SC_EXP = 0.0625 / (S16 * S16 * S16 * S16)   # d^-0.5 / (qp x16 . kp x256)
KV8 = 512 * RG               # bytes of one fp8 kp or vp piece
CTXB = 512 * RG * 2          # bytes of one bf16 ctx piece (rt)
VPB = 512 * RG * 2           # bytes of one bf16 vp piece (rt)

_CACHE = {}


def build_nc():
    nc = bacc.Bacc("TRN2", target_bir_lowering=False, debug=False,
                   num_devices=NCORES)
    dram = {}

    def din(name, shape, dt=BF16):
        dram[name] = nc.dram_tensor(name, shape, dt, kind="ExternalInput").ap()

    din("x_text", [E, E])
    din("x_loc", [E, E])
    din("x_glob", [E, E])
    din("x_loc8", [E, E], FP8)   # pre-scaled x16, raw (K path)
    din("x_glob8", [E, E], FP8)  # pre-scaled x16, raw (K path)
    din("x_text_own", [E, RG])
    din("x_loc_own", [E, RG])
    for w in ("w_tl", "w_tg", "w_rep"):
        din(w, [E, E])
    for b in ("b_tl", "b_tg", "b_rep"):
        din(b, [E], F32)
    # Q/K weights fp8 x16 (scores tolerate it); V/out weights bf16.
    # pos embeddings are folded on the host: the K-side term is softmax-
    # invariant (dropped), the V-side term folds into bv.
    for m in ("tl", "tg"):
        din(f"wq_{m}", [E, E], FP8)
        din(f"wk_{m}", [E, E], FP8)
        din(f"wv_{m}", [E, E])
        din(f"wo_{m}", [E, E])
        din(f"bq_{m}", [E], F32)     # x256
        din(f"bo_{m}", [E], F32)     # x1; bv (incl. pos fold) via wo
    for m in ("ff", "rt"):
        din(f"wq_{m}", [E, 512], FP8)
        din(f"wk_{m}", [E, 512], FP8)
        din(f"wv_{m}", [E, 512])
        din(f"wo_{m}", [E, E])
        din(f"bq_{m}", [512], F32)   # x256
        din(f"bo_{m}", [E], F32)     # x1; bv folded via wo
    out_logits = nc.dram_tensor("logits", [RG, RG], F32,
                                kind="ExternalOutput").ap()
    dbg_names = [x for x in os.environ.get("KDEBUG", "").split(",") if x]

    from contextlib import ExitStack
    with tile.TileContext(nc) as tc, ExitStack() as ctx:
        consts = ctx.enter_context(tc.tile_pool(name="consts", bufs=1))
        streams = ctx.enter_context(tc.tile_pool(name="streams", bufs=2))
        kv8s = ctx.enter_context(tc.tile_pool(name="kv8s", bufs=1))
        kvfull = ctx.enter_context(tc.tile_pool(name="kvfull", bufs=1))
        kvbig = ctx.enter_context(tc.tile_pool(name="kvbig", bufs=1))
        kvbf = ctx.enter_context(tc.tile_pool(name="kvbf", bufs=1))
        wfull = ctx.enter_context(tc.tile_pool(name="wfull", bufs=3))
        wfull8 = ctx.enter_context(tc.tile_pool(name="wfull8", bufs=1))
        whalf8 = ctx.enter_context(tc.tile_pool(name="whalf8", bufs=2))
        acts = ctx.enter_context(tc.tile_pool(name="acts", bufs=4))
        acts8 = ctx.enter_context(tc.tile_pool(name="acts8", bufs=2))
        pers = ctx.enter_context(tc.tile_pool(name="pers", bufs=1))
        qps = ctx.enter_context(tc.tile_pool(name="qps", bufs=1))
        exps = ctx.enter_context(tc.tile_pool(name="exps", bufs=2))
        sqs = ctx.enter_context(tc.tile_pool(name="sqs", bufs=2))
        ctxs = ctx.enter_context(tc.tile_pool(name="ctxs", bufs=1))
        bcs = ctx.enter_context(tc.tile_pool(name="bcs", bufs=1))
        smalls = ctx.enter_context(tc.tile_pool(name="smalls", bufs=1))
        ps512 = ctx.enter_context(tc.tile_pool(name="ps512", bufs=3,
                                               space="PSUM"))
        ps256 = ctx.enter_context(tc.tile_pool(name="ps256", bufs=2,
                                               space="PSUM"))
        pssum = ctx.enter_context(tc.tile_pool(name="pssum", bufs=1,
                                               space="PSUM"))
        dram_p = ctx.enter_context(tc.tile_pool(name="dram_p", bufs=1,
                                                space="DRAM"))

        # ---------- constants ----------
        ones_cb = consts.tile([P, 1], BF16)
        nc.vector.memset(ones_cb, 1.0)
        ones_cf = consts.tile([P, 1], F32)
        nc.vector.memset(ones_cf, 1.0)
        ones_col = consts.tile([P, 1], F32R)
        nc.vector.tensor_copy(ones_col, ones_cf)

        def row_const(val):
            tf = consts.tile([1, P], F32, name=f"rc_{val}")
            nc.vector.memset(tf, val)
            tr = consts.tile([1, P], F32R, name=f"rcr_{val}")
            nc.vector.tensor_copy(tr, tf)
            return tr

        ones_row = row_const(1.0)

        def load_bias_pp(name, n):
            t = consts.tile([P, n // P], F32, name=f"c_{name}")
            nc.sync.dma_start(t, dram[name].rearrange("(c p) -> p c", p=P))
            return t

        # ---------- input streams first (gate the first compute) ----------
        def load_stream(name):
            t = streams.tile([P, KO, E], BF16, tag="x", name=name)
            nc.sync.dma_start(t, dram[name].rearrange("(ko p) r -> p ko r",
                                                      p=P))
            return t

        kvg = load_stream("x_glob")            # becomes globn in place
        kvl = load_stream("x_loc")             # becomes localn in place
        kvg8 = kv8s.tile([P, KO, E], FP8, tag="kv8", name="kvg8")
        nc.sync.dma_start(kvg8, dram["x_glob8"].rearrange(
            "(ko p) r -> p ko r", p=P))
        t_own = pers.tile([P, KO, RG], BF16, name="textn_own")
        nc.sync.dma_start(t_own, dram["x_text_own"].rearrange(
            "(ko p) r -> p ko r", p=P))
        l_own = pers.tile([P, KO, RG], BF16, name="localn_own")
        nc.sync.dma_start(l_own, dram["x_loc_own"].rearrange(
            "(ko p) r -> p ko r", p=P))

        bias_pp = {}
        for nm in ("b_tl", "b_tg", "b_rep"):
            bias_pp[nm] = load_bias_pp(nm, E)
        for m in ("tl", "tg"):
            bias_pp[f"bq_{m}"] = load_bias_pp(f"bq_{m}", E)
            bias_pp[f"bo_{m}"] = load_bias_pp(f"bo_{m}", E)
        for m in ("ff", "rt"):
            bias_pp[f"bq_{m}"] = load_bias_pp(f"bq_{m}", 512)
            bias_pp[f"bo_{m}"] = load_bias_pp(f"bo_{m}", E)

        # ---------- debug ----------
        def _dbg(nm, t):
            if nm not in dbg_names:
                return
            do = nc.dram_tensor(f"dbg_{nm}", [P] + list(t.shape[1:]),
                                t.dtype, kind="ExternalOutput").ap()
            nc.sync.dma_start(do, t)

        # ---------- helpers ----------
        def load_w(name, pool, nco, dt=BF16, pad=None):
            t = pool.tile([P, KO, nco * P], dt, tag="w", name=f"w_{name}",
                          padded_shape=[P, KO, pad * P] if pad else None)
            nc.sync.dma_start(t, dram[name].rearrange("(ko p) c -> p ko c",
                                                      p=P))
            return t

        def colsum_inv(src, nko, with_eps=False):
            """src [128, nko, R] bf16: per-free-column 1/||col||, [1,R] f32r."""
            R = src.shape[2]
            inv = smalls.tile([1, R], F32R, tag=f"inv{R}", name="inv")
            for h in range(0, R, 512):
                w = min(512, R - h)
                ps = pssum.tile([1, w], F32, tag=f"cs{w}", name="ps_cs")
                for ko in range(nko):
                    sq = sqs.tile([P, w], BF16, tag=f"sq{w}", name="sq")
                    nc.scalar.activation(sq, src[:, ko, h:h + w], AF.Square)
                    nc.tensor.matmul(ps, ones_cb, sq, start=(ko == 0),
                                     stop=(ko == nko - 1))
                rec = smalls.tile([1, w], F32, tag=f"rc{w}", name="rec")
                with nc.allow_low_precision(reason="fp32 recip+sqrt"):
                    nc.vector.reciprocal(rec, ps)
                if with_eps:
                    # 1/max(ss, eps^2) == min(1/ss, eps^-2), incl. ss == 0
                    nc.vector.tensor_scalar_min(rec, rec,
                                                1.0 / (EPS * EPS))
                nc.scalar.activation(inv[:, h:h + w], rec, AF.Sqrt)
            return inv

        def bcast_row(row_f32r, n, ones=None):
            """[1, n] f32r -> [128, n] f32 broadcast scaled by the ones val."""
            out = bcs.tile([P, n], F32, tag=f"bc{n}", name="bc")
            for h in range(0, n, 512):
                w = min(512, n - h)
                pool, tag = (ps256, "mm") if w <= 256 else (ps512, "mm512")
                ps = pool.tile([P, w], F32, tag=tag, name="ps_bc")
                nc.tensor.matmul(ps, ones or ones_row, row_f32r[:, h:h + w],
                                 start=True, stop=True)
                nc.vector.tensor_copy(out[:, h:h + w], ps)
            return out

        ADD = mybir.AluOpType.add
        MULT = mybir.AluOpType.mult

        def psum_out(o, ps, bias=None, scale=None, residual=None,
                     act_copy=False):
            """PSUM -> SBUF with optional (x scale), (+ bias), (+ residual)."""
            if bias is not None and residual is not None:
                assert scale is None
                nc.vector.scalar_tensor_tensor(o, ps, bias, residual,
                                               ADD, ADD)
            elif bias is not None and scale is not None:
                # (ps + bias) * scale -- biases ship pre-multiplied so that
                # this fused form lands in the target fp8/bf16 scale
                nc.vector.tensor_scalar(o, ps, bias, scale, ADD, MULT)
            elif bias is not None:
                nc.vector.tensor_scalar_add(o, ps, bias)
            elif scale is not None:
                nc.vector.tensor_scalar_mul(o, ps, scale)
            elif act_copy:
                nc.scalar.copy(o, ps)
            else:
                nc.vector.tensor_copy(o, ps)
            if residual is not None and bias is None:
                nc.gpsimd.tensor_add(o, o, residual)

        def gemm_fm(w_sb, act, out, nco, bias=None, scale=None,
                    residual=None, act_copy=False):
            """bf16 feat-major GEMM: out[:,c,:] = w[:,:,c128].T @ act."""
            R = act.shape[2]
            for c in range(nco):
                for h in range(0, R, 512):
                    w = min(512, R - h)
                    pool, tag = (ps256, "mm") if w <= 256 else (ps512, "mm512")
                    ps = pool.tile([P, w], F32, tag=tag, name="ps_g")
                    for ko in range(KO):
                        nc.tensor.matmul(ps, w_sb[:, ko, c * P:(c + 1) * P],
                                         act[:, ko, h:h + w], start=(ko == 0),
                                         stop=(ko == KO - 1))
                    psum_out(out[:, c, h:h + w], ps,
                             bias[:, c:c + 1] if bias is not None else None,
                             scale,
                             residual[:, c, h:h + w] if residual is not None
                             else None, act_copy)

        def gemm_dr(w8, act8, out, nco, bias=None, scale=None,
                    residual=None, act_copy=False, colscale=None):
            """fp8 DoubleRow GEMM (4x PE): same contract as gemm_fm.
            colscale: [128, R] broadcast tile multiplied per output column
            (commuted norm scaling for the K projections)."""
            R = act8.shape[2]
            for c in range(nco):
                for h in range(0, R, 512):
                    w = min(512, R - h)
                    pool, tag = (ps256, "mm") if w <= 256 else (ps512, "mm512")
                    ps = pool.tile([P, w], F32, tag=tag, name="ps_g")
                    for k2 in range(0, KO, 2):
                        nc.tensor.matmul(ps,
                                         w8[:, k2:k2 + 2, c * P:(c + 1) * P],
                                         act8[:, k2:k2 + 2, h:h + w],
                                         start=(k2 == 0), stop=(k2 == KO - 2),
                                         perf_mode=DRow)
                    if colscale is not None:
                        nc.vector.tensor_mul(out[:, c, h:h + w], ps,
                                             colscale[:, h:h + w])
                        continue
                    psum_out(out[:, c, h:h + w], ps,
                             bias[:, c:c + 1] if bias is not None else None,
                             scale,
                             residual[:, c, h:h + w] if residual is not None
                             else None, act_copy)

        def vproj_smajor(w_sb, act, vp, nchan):
            """bf16 S-major V projection: vp[:,s,:] = act[:,:,s128].T @ w."""
            S = act.shape[2]
            for s in range(S // P):
                for h in range(0, nchan, 512):
                    w = min(512, nchan - h)
                    ps = ps512.tile([P, w], F32, tag="mm512", name="ps_v")
                    for ko in range(KO):
                        nc.tensor.matmul(ps, act[:, ko, s * P:(s + 1) * P],
                                         w_sb[:, ko, h:h + w],
                                         start=(ko == 0), stop=(ko == KO - 1))
                    nc.scalar.copy(vp[:, s, h:h + w], ps)

        def attention_mx(qp8, kp2, vp_sl, ctx_out, nheads):
            """fp8 DoubleRow scores + bf16 denominator/AV/context.
            Heads run in interleaved pairs so the cross-engine exp/recip/
            broadcast latencies hide under the other head's matmuls; the
            pair shares one reciprocal and one broadcast.  The V bias is
            folded into the out-projection bias on the host."""
            for hp in range(0, nheads, 2):
                pss = pssum.tile([1, 2 * RG], F32, tag="cs512", name="ps_sm")
                expts = []
                for i in range(2):
                    h = hp + i
                    expt = exps.tile([P, KO, RG], BF16, tag="expb",
                                     name=f"exptb{i}")
                    for s in range(KO):
                        ps = ps256.tile([P, RG], F32, tag="mm", name="ps_sc")
                        nc.tensor.matmul(ps, kp2(h, s),
                                         qp8[:, 2 * h:2 * h + 2],
                                         start=True, stop=True,
                                         perf_mode=DRow)
                        nc.scalar.activation(expt[:, s], ps, AF.Exp,
                                             scale=SC_EXP)
                    expts.append(expt)
                for i in range(2):
                    for s in range(KO):
                        nc.tensor.matmul(pss[:, i * RG:(i + 1) * RG],
                                         ones_cb, expts[i][:, s],
                                         start=(s == 0), stop=(s == KO - 1))
                inv = smalls.tile([1, 2 * RG], F32R, tag="invsm",
                                  name="inv_sm")
                with nc.allow_low_precision(reason="fp32r rounding intended"):
                    nc.vector.reciprocal(inv, pss)
                bc = bcast_row(inv, 2 * RG)
                for i in range(2):
                    h = hp + i
                    for dk in range(2):
                        cc = 2 * h + dk
                        ps = ps256.tile([P, RG], F32, tag="mm", name="ps_av")
                        for s in range(KO):
                            nc.tensor.matmul(ps, vp_sl(s, cc),
                                             expts[i][:, s],
                                             start=(s == 0),
                                             stop=(s == KO - 1))
                        nc.vector.tensor_mul(ctx_out[:, cc], ps,
                                             bc[:, i * RG:(i + 1) * RG])

        def pack_piece(inbuf, off, sb_tile):
            """SBUF tile -> byte-typed dram flat buffer (uint8 bitcast)."""
            t = sb_tile.bitcast(U8)
            shp = t.shape
            n = P * shp[1] * shp[2]
            nc.sync.dma_start(
                inbuf[off:off + n].rearrange("(p a b) -> p a b", p=P,
                                             a=shp[1]), t)

        def allgather(inbuf, outbuf, groups):
            nc.gpsimd.collective_compute(
                "AllGather", mybir.AluOpType.bypass,
                replica_groups=groups,
                ins=[inbuf.opt()], outs=[outbuf.opt()])

        # ---------- stage 0: normalize ----------
        # glob first (feeds the tg chain = critical path).  The bf16 master
        # is normalized in place (V path); the raw fp8 copy feeds the K
        # projection, whose column norm scaling commutes to the kp output.
        inv_g = colsum_inv(kvg, KO)
        bc_g = bcast_row(inv_g, E)
        for ko in range(KO):
            nc.vector.tensor_mul(kvg[:, ko], kvg[:, ko], bc_g)
        inv_l = colsum_inv(kvl, KO)
        bc_l = bcast_row(inv_l, E)
        for ko in range(KO):
            nc.vector.tensor_mul(kvl[:, ko], kvl[:, ko], bc_l)
        inv_to = colsum_inv(t_own, KO)
        bc_to = bcast_row(inv_to, RG)
        for ko in range(KO):
            nc.vector.tensor_mul(t_own[:, ko], t_own[:, ko], bc_to)
        inv_lo = colsum_inv(l_own, KO)
        bc_lo = bcast_row(inv_lo, RG)
        for ko in range(KO):
            nc.vector.tensor_mul(l_own[:, ko], l_own[:, ko], bc_lo)
        # full-text norms via chunked DMA (all Sqrt before the first Exp)
        inv_t = smalls.tile([1, E], F32R, tag="inv1024", name="inv_t")
        for h in range(2):
            ps = pssum.tile([1, 512], F32, tag="cs512", name="ps_xt")
            for ko in range(KO):
                stg = sqs.tile([P, 512], BF16, tag="stg", name="stg")
                nc.sync.dma_start(
                    stg, dram["x_text"][ko * P:(ko + 1) * P,
                                        h * 512:(h + 1) * 512])
                sq = sqs.tile([P, 512], BF16, tag="sq512", name="sq")
                nc.scalar.activation(sq, stg, AF.Square)
                nc.tensor.matmul(ps, ones_cb, sq, start=(ko == 0),
                                 stop=(ko == KO - 1))
            rec = smalls.tile([1, 512], F32, tag="rc512", name="rec_t")
            with nc.allow_low_precision(reason="fp32 recip+sqrt"):
                nc.vector.reciprocal(rec, ps)
            nc.scalar.activation(inv_t[:, h * 512:(h + 1) * 512], rec,
                                 AF.Sqrt)

        _dbg("kvg8", kvg8)
        _dbg("t_own", t_own)
        _dbg("l_own", l_own)

        # ---------- tg chain (critical path to the first collective) ------
        wk_tg = load_w("wk_tg", wfull8, 8, FP8)
        kp_tg = kvfull.tile([P, KO, E], FP8, tag="kp8", name="kp_tg")
        gemm_dr(wk_tg, kvg8, kp_tg, KO, colscale=bc_g)
        wv_tg = load_w("wv_tg", wfull, 8)
        vp_tg = kvbig.tile([P, KO, E], BF16, tag="vpb", name="vp_tg")
        vproj_smajor(wv_tg, kvg, vp_tg, E)

        w_tg = load_w("w_tg", wfull, 8)
        t_g = acts.tile([P, KO, RG], BF16, tag="act", name="t_g")
        gemm_fm(w_tg, t_own, t_g, KO, bias=bias_pp["b_tg"])
        t_g8 = acts8.tile([P, KO, RG], FP8, tag="act8", name="t_g8")
        for ko in range(KO):
            nc.scalar.activation(t_g8[:, ko], t_g[:, ko], AF.Copy, scale=S16)
        wq_tg = load_w("wq_tg", wfull8, 8, FP8)
        qp_tg = qps.tile([P, KO, RG], FP8, tag="qp", name="qp_tg")
        gemm_dr(wq_tg, t_g8, qp_tg, KO, bias=bias_pp["bq_tg"],
                scale=1.0 / S16)

        ctx_tg = ctxs.tile([P, KO, RG], BF16, tag="cf", name="ctx_tg")
        attention_mx(qp_tg,
                     lambda h, s: kp_tg[:, 2 * h:2 * h + 2,
                                        s * P:(s + 1) * P],
                     lambda s, cc: vp_tg[:, s, cc * P:(cc + 1) * P],
                     ctx_tg, 4)
        wo_tg = load_w("wo_tg", wfull, 8)
        gt = acts.tile([P, KO, RG], BF16, tag="act", name="gt")
        gemm_fm(wo_tg, ctx_tg, gt, KO, bias=bias_pp["bo_tg"], residual=t_g)
        gt8 = acts8.tile([P, KO, RG], FP8, tag="act8", name="gt8")
        for ko in range(KO):
            nc.scalar.activation(gt8[:, ko], gt[:, ko], AF.Copy, scale=S16)

        # ff K/V S-shard + quad AllGather (AG_C) -- issue ASAP
        wk_ff = load_w("wk_ff", whalf8, 4, FP8)
        kp_ff = ctxs.tile([P, 4, RG], FP8, tag="kp", name="kp_ff")
        gemm_dr(wk_ff, gt8, kp_ff, 4, act_copy=True)
        wv_ff = load_w("wv_ff", wfull, 4, pad=8)
        vp_ff = ctxs.tile([P, 2, 512], BF16, tag="vp", name="vp_ff")
        vproj_smajor(wv_ff, gt, vp_ff, 512)
        _dbg("kp_ff_piece", kp_ff)
        _dbg("vp_ff_piece", vp_ff)
        in_ff = dram_p.tile([KV8 + VPB], U8, name="in_ff")
        out_ff = dram_p.tile([4, KV8 + VPB], U8, name="out_ff")
        pack_piece(in_ff, 0, kp_ff)
        pack_piece(in_ff, KV8, vp_ff)
        allgather(in_ff, out_ff, GROUPS4)

        # ---------- tl chain (runs on the PE under AG_C) ----------
        # kvl8 shares kvg8's slab; its DMA is issued only here so the SP
        # queue can't head-of-line block the tg-chain weight loads on it
        kvl8 = kv8s.tile([P, KO, E], FP8, tag="kv8", name="kvl8")
        nc.sync.dma_start(kvl8, dram["x_loc8"].rearrange(
            "(ko p) r -> p ko r", p=P))
        wk_tl = load_w("wk_tl", wfull8, 8, FP8)
        kp_tl = kvfull.tile([P, KO, E], FP8, tag="kp8", name="kp_tl")
        gemm_dr(wk_tl, kvl8, kp_tl, KO, colscale=bc_l)
        wv_tl = load_w("wv_tl", wfull, 8)
        vp_tl = kvbig.tile([P, KO, E], BF16, tag="vpb", name="vp_tl")
        vproj_smajor(wv_tl, kvl, vp_tl, E)

        w_tl = load_w("w_tl", wfull, 8)
        t_l = acts.tile([P, KO, RG], BF16, tag="act", name="t_l")
        gemm_fm(w_tl, t_own, t_l, KO, bias=bias_pp["b_tl"])
        t_l8 = acts8.tile([P, KO, RG], FP8, tag="act8", name="t_l8")
        for ko in range(KO):
            nc.scalar.activation(t_l8[:, ko], t_l[:, ko], AF.Copy, scale=S16)
        wq_tl = load_w("wq_tl", wfull8, 8, FP8)
        qp_tl = qps.tile([P, KO, RG], FP8, tag="qp", name="qp_tl")
        gemm_dr(wq_tl, t_l8, qp_tl, KO, bias=bias_pp["bq_tl"],
                scale=1.0 / S16)

        ctx_tl = ctxs.tile([P, KO, RG], BF16, tag="cf", name="ctx_tl")
        attention_mx(qp_tl,
                     lambda h, s: kp_tl[:, 2 * h:2 * h + 2,
                                        s * P:(s + 1) * P],
                     lambda s, cc: vp_tl[:, s, cc * P:(cc + 1) * P],
                     ctx_tl, 4)
        wo_tl = load_w("wo_tl", wfull, 8)
        lt = acts.tile([P, KO, RG], BF16, tag="act", name="lt")
        gemm_fm(wo_tl, ctx_tl, lt, KO, bias=bias_pp["bo_tl"], residual=t_l)
        lt8 = acts8.tile([P, KO, RG], FP8, tag="act8", name="lt8")
        for ko in range(KO):
            nc.scalar.activation(lt8[:, ko], lt[:, ko], AF.Copy, scale=S16)
        wq_ff = load_w("wq_ff", whalf8, 4, FP8)
        qp_ff = qps.tile([P, 4, RG], FP8, tag="qph", name="qp_ff")
        gemm_dr(wq_ff, lt8, qp_ff, 4, bias=bias_pp["bq_ff"], scale=1.0 / S16)

        _dbg("t_l", t_l)
        _dbg("lt", lt)

        # ---------- ff attention (waits on AG_C) ----------
        kpf_ff = kvfull.tile([P, 4, 4, RG], FP8, tag="kp8", name="kpf_ff",
                             padded_shape=[P, 4, 4, 2 * RG])
        vpf_ff = kvbig.tile([P, 4, 2, 512], BF16, tag="vpb", name="vpf_ff",
                            padded_shape=[P, 4, 2, 1024])
        for gs in range(4):
            nc.sync.dma_start(
                kpf_ff[:, gs].bitcast(U8),
                out_ff[gs, 0:KV8].rearrange("(p a b) -> p a b", p=P, a=4))
            nc.sync.dma_start(
                vpf_ff[:, gs].bitcast(U8),
                out_ff[gs, KV8:].rearrange("(p a b) -> p a b", p=P, a=2))
        _dbg("kpf_ff", kpf_ff)
        ctxh_ff = ctxs.tile([P, 4, RG], BF16, tag="cf", name="ctxh_ff",
                            padded_shape=[P, 4, 2 * RG])
        attention_mx(qp_ff,
                     lambda h, s: kpf_ff[:, s // 2, 2 * h:2 * h + 2,
                                         (s % 2) * P:(s % 2 + 1) * P],
                     lambda s, cc: vpf_ff[:, s // 2, s % 2,
                                          cc * P:(cc + 1) * P],
                     ctxh_ff, 2)
        _dbg("ctxh_ff", ctxh_ff)
        in_cff = dram_p.tile([CTXB], U8, name="in_cff")
        out_cff = dram_p.tile([2, CTXB], U8, name="out_cff")
        pack_piece(in_cff, 0, ctxh_ff)
        allgather(in_cff, out_cff, GROUPS2)

        # t_r (full rows) + qp_rt fill the AG_D window
        xt = load_stream("x_text")
        bc_t = bcast_row(inv_t, E)
        for ko in range(KO):
            nc.vector.tensor_mul(xt[:, ko], xt[:, ko], bc_t)
        w_rep = load_w("w_rep", wfull, 8)
        t_r = kvbig.tile([P, KO, E], BF16, tag="vpb", name="t_r")
        gemm_fm(w_rep, xt, t_r, KO, bias=bias_pp["b_rep"])
        wq_rt = load_w("wq_rt", whalf8, 4, FP8)
        t_r_own8 = acts8.tile([P, KO, RG], FP8, tag="act8", name="t_r_own8")
        gemm_fm(w_rep, t_own, t_r_own8, KO, bias=bias_pp["b_rep"], scale=S16)
        qp_rt = qps.tile([P, 4, RG], FP8, tag="qph", name="qp_rt")
        gemm_dr(wq_rt, t_r_own8, qp_rt, 4, bias=bias_pp["bq_rt"],
                scale=1.0 / S16)

        # ff out-projection + residual -> ff activation (bf16 + fp8 copy)
        ctxf_ff = ctxs.tile([P, KO, RG], BF16, tag="cf", name="ctxf_ff")
        for r in range(2):
            nc.sync.dma_start(
                ctxf_ff[:, 4 * r:4 * r + 4].bitcast(U8),
                out_cff[r].rearrange("(p a b) -> p a b", p=P, a=4))
        wo_ff = load_w("wo_ff", wfull, 8)
        ffa = acts.tile([P, KO, RG], BF16, tag="act", name="ffa")
        gemm_fm(wo_ff, ctxf_ff, ffa, KO, bias=bias_pp["bo_ff"], residual=lt)
        ffa8 = acts8.tile([P, KO, RG], FP8, tag="act8", name="ffa8")
        for ko in range(KO):
            nc.scalar.activation(ffa8[:, ko], ffa[:, ko], AF.Copy, scale=S16)
        _dbg("ffa", ffa)

        # rt K/V S-shard + quad AllGather (AG_E): K fp8, V bf16
        wk_rt = load_w("wk_rt", whalf8, 4, FP8)
        kp_rt = ctxs.tile([P, 4, RG], FP8, tag="kp", name="kp_rt")
        gemm_dr(wk_rt, ffa8, kp_rt, 4, act_copy=True)
        wv_rt = load_w("wv_rt", wfull, 4, pad=8)
        vp_rt = kvbf.tile([P, 2, 512], BF16, tag="rtkv", name="vp_rt",
                          padded_shape=[P, 2, 2048])
        vproj_smajor(wv_rt, ffa, vp_rt, 512)
        in_rt = dram_p.tile([KV8 + VPB], U8, name="in_rt")
        out_rt = dram_p.tile([4, KV8 + VPB], U8, name="out_rt")
        pack_piece(in_rt, 0, kp_rt)
        pack_piece(in_rt, KV8, vp_rt)
        allgather(in_rt, out_rt, GROUPS4)

        # B = wo_rt . t_r and g = t_r.T bo_rt fill the AG_E window, so the
        # post-AG_F tail needs no rt out-projection: full.T = B.T@ctx_rt + g
        wo_rt_fm = load_w("wo_rt", wfull, 8)
        Bm = streams.tile([P, KO, E], BF16, tag="x", name="Bmat")
        gemm_fm(wo_rt_fm, t_r, Bm, KO)
        bo_rt_bf = consts.tile([P, KO], BF16, name="bo_rt_bf")
        nc.vector.tensor_copy(bo_rt_bf, bias_pp["bo_rt"])
        g_pp = consts.tile([P, KO], F32, name="g_pp")
        for nch in range(KO):
            psg = pssum.tile([P, 1], F32, tag="g1", name="ps_g1")
            for ko in range(KO):
                nc.tensor.matmul(psg, t_r[:, ko, nch * P:(nch + 1) * P],
                                 bo_rt_bf[:, ko:ko + 1], start=(ko == 0),
                                 stop=(ko == KO - 1))
            nc.vector.tensor_copy(g_pp[:, nch:nch + 1], psg)

        # ---------- rt attention (waits on AG_E) ----------
        kpf_rt = kvfull.tile([P, 4, 4, RG], FP8, tag="kp8", name="kpf_rt",
                             padded_shape=[P, 4, 4, 2 * RG])
        vpf_rt = kvbf.tile([P, 4, 2, 512], BF16, tag="rtkv",
                           name="vpf_rt")
        for gs in range(4):
            nc.sync.dma_start(
                kpf_rt[:, gs].bitcast(U8),
                out_rt[gs, 0:KV8].rearrange("(p a b) -> p a b", p=P, a=4))
            nc.sync.dma_start(
                vpf_rt[:, gs].bitcast(U8),
                out_rt[gs, KV8:].rearrange("(p a b) -> p a b", p=P, a=2))
        ctxh_rt = acts.tile([P, 4, RG], BF16, tag="act", name="ctxh_rt",
                            padded_shape=[P, 4, 2 * RG])
        attention_mx(qp_rt,
                     lambda h, s: kpf_rt[:, s // 2, 2 * h:2 * h + 2,
                                         (s % 2) * P:(s % 2 + 1) * P],
                     lambda s, cc: vpf_rt[:, s // 2, s % 2,
                                          cc * P:(cc + 1) * P],
                     ctxh_rt, 2)
        _dbg("ctxh_rt", ctxh_rt)
        in_crt = dram_p.tile([CTXB], U8, name="in_crt")
        out_crt = dram_p.tile([2, CTXB], U8, name="out_crt")
        pack_piece(in_crt, 0, ctxh_rt)
        allgather(in_crt, out_crt, GROUPS2)

        ctxf_rt = kvbf.tile([P, KO, RG], BF16, tag="rtkv", name="ctxf_rt",
                            padded_shape=[P, KO, 2 * RG])
        for r in range(2):
            nc.sync.dma_start(
                ctxf_rt[:, 4 * r:4 * r + 4].bitcast(U8),
                out_crt[r].rearrange("(p a b) -> p a b", p=P, a=4))

        # ---------- full.T = B.T @ ctx_rt + g, cosine logits ----------
        fullT = acts.tile([P, KO, RG], BF16, tag="act", name="fullT")
        for nchunk in range(KO):
            ps = ps256.tile([P, RG], F32, tag="mm", name="ps_full")
            for cc in range(KO):
                nc.tensor.matmul(ps, Bm[:, cc, nchunk * P:(nchunk + 1) * P],
                                 ctxf_rt[:, cc], start=(cc == 0),
                                 stop=(cc == KO - 1))
            nc.vector.tensor_scalar_add(fullT[:, nchunk], ps,
                                        g_pp[:, nchunk:nchunk + 1])

        inv_full = colsum_inv(fullT, KO, with_eps=True)
        # transpose the per-row inverse norms to a per-partition layout via
        # a tiny DRAM roundtrip, then fold the cosine normalization into the
        # logits copy-out as a per-partition scale
        invq_d = dram_p.tile([RG], F32, name="invq_d")
        nc.sync.dma_start(invq_d, inv_full.bitcast(F32))
        invq = smalls.tile([P, 2], F32, tag="invq", name="invq")
        nc.sync.dma_start(invq, invq_d.rearrange("(c p) -> p c", p=P))

        lg = bcs.tile([P, 2, RG], F32, tag="bc1024", name="lg",
                      padded_shape=[P, 2, 2 * RG])
        for lc in range(2):
            ps = ps256.tile([P, RG], F32, tag="mm", name="ps_lg")
            for ko in range(KO):
                nc.tensor.matmul(ps, fullT[:, ko, lc * P:(lc + 1) * P],
                                 l_own[:, ko], start=(ko == 0),
                                 stop=(ko == KO - 1))
            nc.vector.tensor_scalar_mul(lg[:, lc], ps, invq[:, lc:lc + 1])
        nc.sync.dma_start(out_logits.rearrange("(lc p) q -> p lc q", p=P), lg)

    nc.compile()
    return nc


def make_in_maps(local_feat, global_feat, text_feat,
                 w_tl, b_tl, w_tg, b_tg, w_rep, b_rep,
                 pos_local, pos_global, mha_params):
    """mha_params: dict m -> (wi, bi, wo, bo)."""
    f32 = np.float32
    bf16 = ml_dtypes.bfloat16
    fp8 = ml_dtypes.float8_e4m3

    def tb(x):
        return np.ascontiguousarray(np.asarray(x).T.astype(bf16))

    def t8(x):
        return np.ascontiguousarray(
            (np.asarray(x).T.astype(f32) * 16.0).astype(fp8))

    textT = tb(text_feat)
    locT = tb(local_feat)
    shared = {
        "x_text": textT, "x_loc": locT, "x_glob": tb(global_feat),
        "x_loc8": t8(local_feat), "x_glob8": t8(global_feat),
        "w_tl": tb(w_tl), "w_tg": tb(w_tg), "w_rep": tb(w_rep),
        "b_tl": b_tl.astype(f32), "b_tg": b_tg.astype(f32),
        "b_rep": b_rep.astype(f32),
    }
    pos = {"tl": np.asarray(pos_local, dtype=np.float64),
           "tg": np.asarray(pos_global, dtype=np.float64)}
    for m in ("tl", "tg"):
        wi, bi, wo, bo = mha_params[m]
        shared[f"wq_{m}"] = t8(wi[0 * E:1 * E])
        shared[f"wk_{m}"] = t8(wi[1 * E:2 * E])
        shared[f"wv_{m}"] = tb(wi[2 * E:3 * E])
        shared[f"wo_{m}"] = tb(wo)
        shared[f"bq_{m}"] = bi[0 * E:1 * E].astype(f32) * 256.0
        # parameter-only folds: the V-side pos term joins bv, and the
        # whole V bias is pushed through the out-projection into bo; the
        # K-side pos term is softmax-invariant and dropped
        bv_fold = (np.asarray(bi[2 * E:3 * E], dtype=np.float64)
                   + pos[m] @ np.asarray(wi[2 * E:3 * E],
                                         dtype=np.float64).T)
        shared[f"bo_{m}"] = (np.asarray(bo, dtype=np.float64)
                             + bv_fold
                             @ np.asarray(wo, dtype=np.float64).T
                             ).astype(f32)
    per_j = {}
    for j in range(2):
        d = {}
        sl = slice(512 * j, 512 * (j + 1))
        for m in ("ff", "rt"):
            wi, bi, wo, bo = mha_params[m]
            d[f"wq_{m}"] = t8(wi[0 * E:1 * E][sl])
            d[f"wk_{m}"] = t8(wi[1 * E:2 * E][sl])
            d[f"wv_{m}"] = tb(wi[2 * E:3 * E][sl])
            if m == "rt":
                # wo_rt is consumed as B = wo_rt . t_r (contraction over
                # the output-feature axis), so it ships untransposed
                d[f"wo_{m}"] = np.ascontiguousarray(
                    np.asarray(wo).astype(bf16))
            else:
                d[f"wo_{m}"] = tb(wo)
            d[f"bq_{m}"] = bi[0 * E:1 * E][sl].astype(f32) * 256.0
            d[f"bo_{m}"] = (np.asarray(bo, dtype=np.float64)
                            + np.asarray(bi[2 * E:3 * E], dtype=np.float64)
                            @ np.asarray(wo, dtype=np.float64).T
                            ).astype(f32)
        per_j[j] = d

    in_maps = []
    for c in range(NCORES):
        g, j = c // 2, c % 2
        rs = slice(RG * g, RG * (g + 1))
        m = {
            "x_text_own": np.ascontiguousarray(textT[:, rs]),
            "x_loc_own": np.ascontiguousarray(locT[:, rs]),
        }
        m.update(shared)
        m.update(per_j[j])
        in_maps.append(m)
    return in_maps


def kernel(local_feat, global_feat, text_feat,
           w_tl, b_tl, w_tg, b_tg, w_rep, b_rep,
           pos_local, pos_global,
           tl_wi, tl_bi, tl_wo, tl_bo,
           tg_wi, tg_bi, tg_wo, tg_bo,
           ff_wi, ff_bi, ff_wo, ff_bo,
           rt_wi, rt_bi, rt_wo, rt_bo,
           n_groups):
    assert int(n_groups) == 4
    if "nc" not in _CACHE:
        _CACHE["nc"] = build_nc()
    nc = _CACHE["nc"]
    mha_params = {
        "tl": (tl_wi, tl_bi, tl_wo, tl_bo),
        "tg": (tg_wi, tg_bi, tg_wo, tg_bo),
        "ff": (ff_wi, ff_bi, ff_wo, ff_bo),
        "rt": (rt_wi, rt_bi, rt_wo, rt_bo),
    }
    in_maps = make_in_maps(np.asarray(local_feat), np.asarray(global_feat),
                           np.asarray(text_feat),
                           np.asarray(w_tl), np.asarray(b_tl),
                           np.asarray(w_tg), np.asarray(b_tg),
                           np.asarray(w_rep), np.asarray(b_rep),
                           np.asarray(pos_local), np.asarray(pos_global),
                           {k: tuple(np.asarray(x) for x in v)
                            for k, v in mha_params.items()})
    res = run_bass_kernel_spmd(nc, in_maps, core_ids=list(range(NCORES)))
    _CACHE["last_results"] = res
    out = np.empty((4, RG, RG), dtype=np.float32)
    for g in range(4):
        out[g] = res.results[2 * g]["logits"]
    return out
